# revision 3
# baseline (speedup 1.0000x reference)
"""Multi-head attention (B=8, N=1024, C=768, H=12) on 8 TRN2 NeuronCores.

Sharding: pure data-parallel over batch — core b computes attention for x[b].
No collectives needed. Per-core Bass/Tile kernel, bf16 compute, f32 PSUM.

Layout strategy (all transposes done on host, for free):
  xT  = x[b].T                    [768, 1024]  (c on partitions)
  wqk = qkv_w[:1536].T            [768, 1536]  (c on partitions)
  wv  = qkv_w[1536:].T            [768, 768]
  wp  = proj_w.T                  [768, 768]   (o on partitions)
  pb  = proj_b.reshape(6,128).T   [128, 6]

Device compute per core:
  qkT[o, n]  = wqk.T @ xT         [1536, 1024] (q rows 0:768, k rows 768:1536;
                                   head h occupies partition rows h*64 mod 128
                                   of o-tile h//2; heads are pair-stacked)
  v[n, o]    = xT.T @ wv          [1024, 768]  (natural layout, stored with a
                                   fused ones-column per head -> [nt, h, 65])
  per head:  S^T[m, n] = k q^T (K=64 matmul, two heads row-packed in PE array)
             E = exp(S^T * scale)            (ScalarE, no max subtraction --
                                              |S*scale| <= ~6 for randn inputs)
             O^T[d, n], sums[n] = [v|1].T @ E  (K=128 accumulate over m-tiles;
                                              row 64 = softmax denominators)
             Onorm^T = O^T * (1/sums) broadcast (reciprocal + gpsimd
                                              partition_broadcast)
  yT[o', n]  = wp.T @ Onorm^T + pb           [768, 1024]
Host: out[b] = yT.T
"""

import numpy as np
import ml_dtypes

B, N, C = 8, 1024, 768
H, D = 12, 64
SCALE = D ** -0.5
CT = C // 128       # 6 contraction tiles
OT = 2 * C // 128   # 12 o-tiles of qkT
NT = N // 128       # 8 token tiles
NCH = N // 512      # 2 n-chunks of 512
HP = H // 2         # 6 head pairs

_CACHE = {}


def _build_nc():
    import concourse.bacc as bacc
    import concourse.mybir as mybir
    import concourse.tile as tile

    f32 = mybir.dt.float32
    bf16 = mybir.dt.bfloat16

    nc = bacc.Bacc("TRN2", target_bir_lowering=False, debug=False, num_devices=8)

    xT_d = nc.dram_tensor("xT", [C, N], bf16, kind="ExternalInput").ap()
    wqk_d = nc.dram_tensor("wqk", [C, 2 * C], bf16, kind="ExternalInput").ap()
    wv_d = nc.dram_tensor("wv", [C, C], bf16, kind="ExternalInput").ap()
    wp_d = nc.dram_tensor("wp", [C, C], bf16, kind="ExternalInput").ap()
    pb_d = nc.dram_tensor("pb", [128, CT], f32, kind="ExternalInput").ap()
    out_d = nc.dram_tensor("out", [C, N], f32, kind="ExternalOutput").ap()

    with tile.TileContext(nc) as tc:
        with (
            tc.tile_pool(name="const", bufs=1) as cpool,
            tc.tile_pool(name="E", bufs=3) as epool,
            tc.tile_pool(name="small", bufs=2) as spool,
            tc.tile_pool(name="y", bufs=2) as ypool,
            tc.tile_pool(name="ps", bufs=4, space="PSUM") as pspool,
        ):
            # ---- persistent SBUF tensors ----
            xT_sb = cpool.tile([128, CT, N], bf16)            # 12KB/part
            wqk_sb = cpool.tile([128, CT, 2 * C], bf16)       # 18KB
            wv_sb = cpool.tile([128, CT, C], bf16)            # 9KB
            wp_sb = cpool.tile([128, CT, C], bf16)            # 9KB
            pb_sb = cpool.tile([128, CT], f32)
            qkT_sb = cpool.tile([128, OT, NCH, 512], bf16)    # 24KB
            v_sb = cpool.tile([128, NT, H, D + 1], bf16)      # 12.2KB
            on_sb = cpool.tile([128, CT, NCH, 512], bf16)     # Onorm^T, 12KB

            for kt in range(CT):
                nc.sync.dma_start(xT_sb[:, kt, :], xT_d[kt * 128:(kt + 1) * 128, :])
                nc.sync.dma_start(wqk_sb[:, kt, :], wqk_d[kt * 128:(kt + 1) * 128, :])
                nc.sync.dma_start(wv_sb[:, kt, :], wv_d[kt * 128:(kt + 1) * 128, :])
                nc.sync.dma_start(wp_sb[:, kt, :], wp_d[kt * 128:(kt + 1) * 128, :])
            nc.sync.dma_start(pb_sb[:], pb_d[:])
            # ones column fused into v (gives softmax sums as O^T row 64)
            nc.vector.memset(v_sb[:, :, :, D:D + 1], 1.0)

            # ---- qkT = wqk.T @ xT  (o on partitions) ----
            for ot in range(OT):
                ps = pspool.tile([128, NCH, 512], f32, tag="ps")
                for nch in range(NCH):
                    for kt in range(CT):
                        nc.tensor.matmul(
                            ps[:, nch, :],
                            wqk_sb[:, kt, ot * 128:(ot + 1) * 128],
                            xT_sb[:, kt, nch * 512:(nch + 1) * 512],
                            start=(kt == 0), stop=(kt == CT - 1),
                        )
                nc.vector.tensor_copy(qkT_sb[:, ot, :, :], ps[:, :, :])

            # ---- v = xT.T @ wv  (natural layout, strided into [nt, h, 65]) ----
            for nt in range(NT):
                ps = pspool.tile([128, 2, 8, 64], f32, tag="ps")
                for och in range(2):
                    for kt in range(CT):
                        nc.tensor.matmul(
                            ps[:, och, 0:6, :],
                            xT_sb[:, kt, nt * 128:(nt + 1) * 128],
                            wv_sb[:, kt, och * 384:(och + 1) * 384],
                            start=(kt == 0), stop=(kt == CT - 1),
                        )
                    nc.vector.tensor_copy(
                        v_sb[:, nt, och * 6:(och + 1) * 6, 0:D], ps[:, och, 0:6, :]
                    )

            # ---- attention, one head pair at a time ----
            for hp in range(HP):
                a, b = 2 * hp, 2 * hp + 1
                E_a = epool.tile([128, NT, NCH, 512], bf16, tag="E")
                E_b = epool.tile([128, NT, NCH, 512], bf16, tag="E")
                # S^T = k @ q^T per m-tile; exp into E
                for mt in range(NT):
                    ps_sa = pspool.tile([128, NCH, 512], f32, tag="ps")
                    ps_sb = pspool.tile([128, NCH, 512], f32, tag="ps")
                    lo = qkT_sb[0:64, 6 + hp, mt // 4, (mt % 4) * 128:(mt % 4 + 1) * 128]
                    hi = qkT_sb[64:128, 6 + hp, mt // 4, (mt % 4) * 128:(mt % 4 + 1) * 128]
                    for nch in range(NCH):
                        nc.tensor.matmul(
                            ps_sa[:, nch, :], lo, qkT_sb[0:64, hp, nch, :],
                            start=True, stop=True,
                        )
                        nc.tensor.matmul(
                            ps_sb[:, nch, :], hi, qkT_sb[64:128, hp, nch, :],
                            start=True, stop=True,
                        )
                    import concourse.mybir as mybir2
                    nc.scalar.activation(
                        E_a[:, mt, :, :], ps_sa[:, :, :],
                        mybir2.ActivationFunctionType.Exp, scale=SCALE,
                    )
                    nc.scalar.activation(
                        E_b[:, mt, :, :], ps_sb[:, :, :],
                        mybir2.ActivationFunctionType.Exp, scale=SCALE,
                    )
                # O^T = [v|1].T @ E, accumulated over m-tiles
                ps_oa = pspool.tile([65, NCH, 512], f32, tag="ps")
                ps_ob = pspool.tile([65, NCH, 512], f32, tag="ps")
                for mt in range(NT):
                    for nch in range(NCH):
                        nc.tensor.matmul(
                            ps_oa[:, nch, :], v_sb[:, mt, a, :], E_a[:, mt, nch, :],
                            start=(mt == 0), stop=(mt == NT - 1),
                        )
                        nc.tensor.matmul(
                            ps_ob[:, nch, :], v_sb[:, mt, b, :], E_b[:, mt, nch, :],
                            start=(mt == 0), stop=(mt == NT - 1),
                        )
                # normalize: row 64 holds the softmax denominators
                rec_a = spool.tile([1, NCH, 512], f32, tag="rec")
                rec_b = spool.tile([1, NCH, 512], f32, tag="rec")
                nc.vector.reciprocal(rec_a[:], ps_oa[64:65, :, :])
                nc.vector.reciprocal(rec_b[:], ps_ob[64:65, :, :])
                R_a = spool.tile([64, NCH, 512], f32, tag="R")
                R_b = spool.tile([64, NCH, 512], f32, tag="R")
                nc.gpsimd.partition_broadcast(R_a[:], rec_a[:])
                nc.gpsimd.partition_broadcast(R_b[:], rec_b[:])
                import concourse.mybir as mybir3
                nc.vector.tensor_tensor(
                    on_sb[0:64, hp, :, :], ps_oa[0:64, :, :], R_a[:],
                    op=mybir3.AluOpType.mult,
                )
                onb = spool.tile([64, NCH, 512], bf16, tag="onb")
                nc.vector.tensor_tensor(
                    onb[:], ps_ob[0:64, :, :], R_b[:], op=mybir3.AluOpType.mult,
                )
                # head b lives at partitions 64:128 -> shift via SBUF->SBUF DMA
                nc.sync.dma_start(on_sb[64:128, hp, :, :], onb[:])

            # ---- yT = wp.T @ Onorm^T + pb ----
            for otp in range(CT):
                ps = pspool.tile([128, NCH, 512], f32, tag="ps")
                for nch in range(NCH):
                    for kt in range(CT):
                        nc.tensor.matmul(
                            ps[:, nch, :],
                            wp_sb[:, kt, otp * 128:(otp + 1) * 128],
                            on_sb[:, kt, nch, :],
                            start=(kt == 0), stop=(kt == CT - 1),
                        )
                yt = ypool.tile([128, NCH, 512], f32, tag="yt")
                nc.vector.tensor_scalar_add(yt[:], ps[:, :, :], pb_sb[:, otp:otp + 1])
                nc.sync.dma_start(out_d[otp * 128:(otp + 1) * 128, :], yt[:])

    nc.compile()
    return nc


def _get_nc():
    if "nc" not in _CACHE:
        _CACHE["nc"] = _build_nc()
    return _CACHE["nc"]


def kernel(x, qkv_w, proj_w, proj_b):
    from concourse.bass_utils import run_bass_kernel_spmd

    nc = _get_nc()
    bf = ml_dtypes.bfloat16
    wqk = np.ascontiguousarray(qkv_w[:2 * C].T).astype(bf)
    wv = np.ascontiguousarray(qkv_w[2 * C:].T).astype(bf)
    wp = np.ascontiguousarray(proj_w.T).astype(bf)
    pb = np.ascontiguousarray(proj_b.reshape(CT, 128).T).astype(np.float32)
    in_maps = []
    for i in range(B):
        in_maps.append({
            "xT": np.ascontiguousarray(x[i].T).astype(bf),
            "wqk": wqk, "wv": wv, "wp": wp, "pb": pb,
        })
    res = run_bass_kernel_spmd(nc, in_maps, core_ids=list(range(B)))
    out = np.stack([res.results[i]["out"].T for i in range(B)])
    return np.ascontiguousarray(out.astype(np.float32))


# revision 6
# speedup vs baseline: 1.1117x; 1.1117x over previous
"""Multi-head attention (B=8, N=1024, C=768, H=12) on 8 TRN2 NeuronCores.

Sharding: pure data-parallel over batch — core b computes attention for x[b].
No collectives needed. Per-core Bass/Tile kernel, bf16 compute, f32 PSUM.

Layout strategy (all transposes done on host, for free):
  xT  = x[b].T                    [768, 1024]  (c on partitions)
  wqk = qkv_w[:1536].T            [768, 1536]  (c on partitions)
  wv  = qkv_w[1536:].T            [768, 768]
  wp  = proj_w.T                  [768, 768]   (o on partitions)
  pb  = proj_b.reshape(6,128).T   [128, 6]

Device compute per core:
  qkT[o, n]  = wqk.T @ xT         [1536, 1024] (q rows 0:768, k rows 768:1536;
                                   head h occupies partition rows h*64 mod 128
                                   of o-tile h//2; heads are pair-stacked)
  v[n, o]    = xT.T @ wv          [1024, 768]  (natural layout, stored with a
                                   fused ones-column per head -> [nt, h, 65])
  per head:  S^T[m, n] = k q^T (K=64 matmul, two heads row-packed in PE array)
             E = exp(S^T * scale)            (ScalarE, no max subtraction --
                                              |S*scale| <= ~6 for randn inputs)
             O^T[d, n], sums[n] = [v|1].T @ E  (K=128 accumulate over m-tiles;
                                              row 64 = softmax denominators)
             Onorm^T = O^T * (1/sums) broadcast (reciprocal + gpsimd
                                              partition_broadcast)
  yT[o', n]  = wp.T @ Onorm^T + pb           [768, 1024]
Host: out[b] = yT.T
"""

import numpy as np
import ml_dtypes

B, N, C = 8, 1024, 768
H, D = 12, 64
SCALE = D ** -0.5
CT = C // 128       # 6 contraction tiles
OT = 2 * C // 128   # 12 o-tiles of qkT
NT = N // 128       # 8 token tiles
NCH = N // 512      # 2 n-chunks of 512
HP = H // 2         # 6 head pairs

_CACHE = {}


def _build_nc():
    import concourse.bacc as bacc
    import concourse.mybir as mybir
    import concourse.tile as tile

    f32 = mybir.dt.float32
    bf16 = mybir.dt.bfloat16

    nc = bacc.Bacc("TRN2", target_bir_lowering=False, debug=False, num_devices=8)

    xT_d = nc.dram_tensor("xT", [C, N], bf16, kind="ExternalInput").ap()
    wqk_d = nc.dram_tensor("wqk", [C, 2 * C], bf16, kind="ExternalInput").ap()
    wv_d = nc.dram_tensor("wv", [C, C], bf16, kind="ExternalInput").ap()
    wp_d = nc.dram_tensor("wp", [C, C], bf16, kind="ExternalInput").ap()
    pb_d = nc.dram_tensor("pb", [128, CT], f32, kind="ExternalInput").ap()
    out_d = nc.dram_tensor("out", [C, N], f32, kind="ExternalOutput").ap()

    with tile.TileContext(nc) as tc:
        with (
            tc.tile_pool(name="const", bufs=1) as cpool,
            tc.tile_pool(name="E", bufs=3) as epool,
            tc.tile_pool(name="small", bufs=2) as spool,
            tc.tile_pool(name="y", bufs=2) as ypool,
            tc.tile_pool(name="ps", bufs=4, space="PSUM") as pspool,
        ):
            # ---- persistent SBUF tensors ----
            xT_sb = cpool.tile([128, CT, N], bf16)            # 12KB/part
            wqk_sb = cpool.tile([128, CT, 2 * C], bf16)       # 18KB
            wv_sb = cpool.tile([128, CT, C], bf16)            # 9KB
            wp_sb = cpool.tile([128, CT, C], bf16)            # 9KB
            pb_sb = cpool.tile([128, CT], f32)
            qkT_sb = cpool.tile([128, OT, NCH, 512], bf16)    # 24KB
            v_sb = cpool.tile([128, NT, H, D + 1], bf16)      # 12.2KB
            on_sb = cpool.tile([128, CT, NCH, 512], bf16)     # Onorm^T, 12KB

            for kt in range(CT):
                nc.sync.dma_start(xT_sb[:, kt, :], xT_d[kt * 128:(kt + 1) * 128, :])
                nc.sync.dma_start(wqk_sb[:, kt, :], wqk_d[kt * 128:(kt + 1) * 128, :])
                nc.sync.dma_start(wv_sb[:, kt, :], wv_d[kt * 128:(kt + 1) * 128, :])
                nc.sync.dma_start(wp_sb[:, kt, :], wp_d[kt * 128:(kt + 1) * 128, :])
            nc.sync.dma_start(pb_sb[:], pb_d[:])
            # ones column fused into v (gives softmax sums as O^T row 64)
            nc.vector.memset(v_sb[:, :, :, D:D + 1], 1.0)

            def emit_qkT(ot):
                ps = pspool.tile([128, NCH, 512], f32, tag="ps")
                for nch in range(NCH):
                    for kt in range(CT):
                        nc.tensor.matmul(
                            ps[:, nch, :],
                            wqk_sb[:, kt, ot * 128:(ot + 1) * 128],
                            xT_sb[:, kt, nch * 512:(nch + 1) * 512],
                            start=(kt == 0), stop=(kt == CT - 1),
                        )
                nc.vector.tensor_copy(qkT_sb[:, ot, :, :], ps[:, :, :])

            def emit_v(nt):
                ps = pspool.tile([128, 2, 8, 64], f32, tag="ps")
                for och in range(2):
                    for kt in range(CT):
                        nc.tensor.matmul(
                            ps[:, och, 0:6, :],
                            xT_sb[:, kt, nt * 128:(nt + 1) * 128],
                            wv_sb[:, kt, och * 384:(och + 1) * 384],
                            start=(kt == 0), stop=(kt == CT - 1),
                        )
                    nc.vector.tensor_copy(
                        v_sb[:, nt, och * 6:(och + 1) * 6, 0:D], ps[:, och, 0:6, :]
                    )

            # ---- attention, one head pair at a time; qkT/v emitted just in
            # time so the exp stream starts early and PE fills ACT-bound gaps
            for hp in range(HP):
                a, b = 2 * hp, 2 * hp + 1
                emit_qkT(hp)
                emit_qkT(6 + hp)
                if hp == 0:
                    for nt in range(NT):
                        emit_v(nt)
                E_a = epool.tile([128, NT, NCH, 512], bf16, tag="E")
                E_b = epool.tile([128, NT, NCH, 512], bf16, tag="E")
                # S^T = k @ q^T per m-tile; exp into E
                for mt in range(NT):
                    ps_sa = pspool.tile([128, NCH, 512], f32, tag="ps")
                    ps_sb = pspool.tile([128, NCH, 512], f32, tag="ps")
                    lo = qkT_sb[0:64, 6 + hp, mt // 4, (mt % 4) * 128:(mt % 4 + 1) * 128]
                    hi = qkT_sb[64:128, 6 + hp, mt // 4, (mt % 4) * 128:(mt % 4 + 1) * 128]
                    for nch in range(NCH):
                        nc.tensor.matmul(
                            ps_sa[:, nch, :], lo, qkT_sb[0:64, hp, nch, :],
                            start=True, stop=True,
                        )
                        nc.tensor.matmul(
                            ps_sb[:, nch, :], hi, qkT_sb[64:128, hp, nch, :],
                            start=True, stop=True,
                        )
                    import concourse.mybir as mybir2
                    nc.scalar.activation(
                        E_a[:, mt, :, :], ps_sa[:, :, :],
                        mybir2.ActivationFunctionType.Exp, scale=SCALE,
                    )
                    nc.scalar.activation(
                        E_b[:, mt, :, :], ps_sb[:, :, :],
                        mybir2.ActivationFunctionType.Exp, scale=SCALE,
                    )
                # O^T = [v|1].T @ E, accumulated over m-tiles
                ps_oa = pspool.tile([65, NCH, 512], f32, tag="ps")
                ps_ob = pspool.tile([65, NCH, 512], f32, tag="ps")
                for mt in range(NT):
                    for nch in range(NCH):
                        nc.tensor.matmul(
                            ps_oa[:, nch, :], v_sb[:, mt, a, :], E_a[:, mt, nch, :],
                            start=(mt == 0), stop=(mt == NT - 1),
                        )
                        nc.tensor.matmul(
                            ps_ob[:, nch, :], v_sb[:, mt, b, :], E_b[:, mt, nch, :],
                            start=(mt == 0), stop=(mt == NT - 1),
                        )
                # normalize: row 64 holds the softmax denominators
                rec_a = spool.tile([1, NCH, 512], f32, tag="rec")
                rec_b = spool.tile([1, NCH, 512], f32, tag="rec")
                sum_a = spool.tile([1, NCH, 512], f32, tag="sum")
                sum_b = spool.tile([1, NCH, 512], f32, tag="sum")
                nc.vector.tensor_copy(sum_a[:], ps_oa[64:65, :, :])
                nc.vector.tensor_copy(sum_b[:], ps_ob[64:65, :, :])
                nc.vector.reciprocal_approx_fast(rec_a[:], sum_a[:])
                nc.vector.reciprocal_approx_fast(rec_b[:], sum_b[:])
                R_a = spool.tile([64, NCH, 512], f32, tag="R")
                R_b = spool.tile([64, NCH, 512], f32, tag="R")
                nc.gpsimd.partition_broadcast(R_a[:], rec_a[:])
                nc.gpsimd.partition_broadcast(R_b[:], rec_b[:])
                import concourse.mybir as mybir3
                nc.vector.tensor_tensor(
                    on_sb[0:64, hp, :, :], ps_oa[0:64, :, :], R_a[:],
                    op=mybir3.AluOpType.mult,
                )
                onb = spool.tile([64, NCH, 512], bf16, tag="onb")
                nc.vector.tensor_tensor(
                    onb[:], ps_ob[0:64, :, :], R_b[:], op=mybir3.AluOpType.mult,
                )
                # head b lives at partitions 64:128 -> shift via SBUF->SBUF DMA
                nc.sync.dma_start(on_sb[64:128, hp, :, :], onb[:])

            # ---- yT = wp.T @ Onorm^T + pb ----
            for otp in range(CT):
                ps = pspool.tile([128, NCH, 512], f32, tag="ps")
                for nch in range(NCH):
                    for kt in range(CT):
                        nc.tensor.matmul(
                            ps[:, nch, :],
                            wp_sb[:, kt, otp * 128:(otp + 1) * 128],
                            on_sb[:, kt, nch, :],
                            start=(kt == 0), stop=(kt == CT - 1),
                        )
                yt = ypool.tile([128, NCH, 512], f32, tag="yt")
                nc.vector.tensor_scalar_add(yt[:], ps[:, :, :], pb_sb[:, otp:otp + 1])
                nc.sync.dma_start(out_d[otp * 128:(otp + 1) * 128, :], yt[:])

    nc.compile()
    return nc


def _get_nc():
    if "nc" not in _CACHE:
        _CACHE["nc"] = _build_nc()
    return _CACHE["nc"]


def kernel(x, qkv_w, proj_w, proj_b):
    from concourse.bass_utils import run_bass_kernel_spmd

    nc = _get_nc()
    bf = ml_dtypes.bfloat16
    wqk = np.ascontiguousarray(qkv_w[:2 * C].T).astype(bf)
    wv = np.ascontiguousarray(qkv_w[2 * C:].T).astype(bf)
    wp = np.ascontiguousarray(proj_w.T).astype(bf)
    pb = np.ascontiguousarray(proj_b.reshape(CT, 128).T).astype(np.float32)
    in_maps = []
    for i in range(B):
        in_maps.append({
            "xT": np.ascontiguousarray(x[i].T).astype(bf),
            "wqk": wqk, "wv": wv, "wp": wp, "pb": pb,
        })
    res = run_bass_kernel_spmd(nc, in_maps, core_ids=list(range(B)))
    out = np.stack([res.results[i]["out"].T for i in range(B)])
    return np.ascontiguousarray(out.astype(np.float32))


# revision 8
# speedup vs baseline: 1.2142x; 1.0922x over previous
"""Multi-head attention (B=8, N=1024, C=768, H=12) on 8 TRN2 NeuronCores.

Sharding: pure data-parallel over batch — core b computes attention for x[b].
No collectives needed. Per-core Bass/Tile kernel, bf16 compute, f32 PSUM.

Layout strategy (all transposes done on host, for free):
  xT  = x[b].T                    [768, 1024]  (c on partitions)
  wqk = qkv_w[:1536].T            [768, 1536]  (c on partitions)
  wv  = qkv_w[1536:].T            [768, 768]
  wp  = proj_w.T                  [768, 768]   (o on partitions)
  pb  = proj_b.reshape(6,128).T   [128, 6]

Device compute per core:
  qkT[o, n]  = wqk.T @ xT         [1536, 1024] (q rows 0:768, k rows 768:1536;
                                   head h occupies partition rows h*64 mod 128
                                   of o-tile h//2; heads are pair-stacked)
  v[n, o]    = xT.T @ wv          [1024, 768]  (natural layout, stored with a
                                   fused ones-column per head -> [nt, h, 65])
  per head:  S^T[m, n] = k q^T (K=64 matmul, two heads row-packed in PE array)
             E = exp(S^T * scale)            (ScalarE, no max subtraction --
                                              |S*scale| <= ~6 for randn inputs)
             O^T[d, n], sums[n] = [v|1].T @ E  (K=128 accumulate over m-tiles;
                                              row 64 = softmax denominators)
             Onorm^T = O^T * (1/sums) broadcast (reciprocal + gpsimd
                                              partition_broadcast)
  yT[o', n]  = wp.T @ Onorm^T + pb           [768, 1024]
Host: out[b] = yT.T
"""

import numpy as np
import ml_dtypes

B, N, C = 8, 1024, 768
H, D = 12, 64
SCALE = D ** -0.5
CT = C // 128       # 6 contraction tiles
OT = 2 * C // 128   # 12 o-tiles of qkT
NT = N // 128       # 8 token tiles
NCH = N // 512      # 2 n-chunks of 512
HP = H // 2         # 6 head pairs

_CACHE = {}


def _build_nc():
    import concourse.bacc as bacc
    import concourse.mybir as mybir
    import concourse.tile as tile

    f32 = mybir.dt.float32
    bf16 = mybir.dt.bfloat16

    nc = bacc.Bacc("TRN2", target_bir_lowering=False, debug=False, num_devices=8)

    xT_d = nc.dram_tensor("xT", [C, N], bf16, kind="ExternalInput").ap()
    wqk_d = nc.dram_tensor("wqk", [C, 2 * C], bf16, kind="ExternalInput").ap()
    wv_d = nc.dram_tensor("wv", [C, C], bf16, kind="ExternalInput").ap()
    wp_d = nc.dram_tensor("wp", [C, C], bf16, kind="ExternalInput").ap()
    pb_d = nc.dram_tensor("pb", [128, CT], f32, kind="ExternalInput").ap()
    out_d = nc.dram_tensor("out", [C, N], f32, kind="ExternalOutput").ap()

    with tile.TileContext(nc) as tc:
        with (
            tc.tile_pool(name="const", bufs=1) as cpool,
            tc.tile_pool(name="E", bufs=3) as epool,
            tc.tile_pool(name="small", bufs=2) as spool,
            tc.tile_pool(name="y", bufs=2) as ypool,
            tc.tile_pool(name="ps", bufs=4, space="PSUM") as pspool,
        ):
            # ---- persistent SBUF tensors ----
            xT_sb = cpool.tile([128, CT, N], bf16)            # 12KB/part
            wqk_sb = cpool.tile([128, CT, 2 * C], bf16)       # 18KB
            wv_sb = cpool.tile([128, CT, C], bf16)            # 9KB
            wp_sb = cpool.tile([128, CT, C], bf16)            # 9KB
            pb_sb = cpool.tile([128, CT], f32)
            qkT_sb = cpool.tile([128, OT, NCH, 512], bf16)    # 24KB
            v_sb = cpool.tile([128, NT, H, D + 1], bf16)      # 12.2KB
            on_sb = cpool.tile([128, CT, NCH, 512], bf16)     # Onorm^T, 12KB

            for kt in range(CT):
                nc.sync.dma_start(xT_sb[:, kt, :], xT_d[kt * 128:(kt + 1) * 128, :])
                nc.sync.dma_start(wqk_sb[:, kt, :], wqk_d[kt * 128:(kt + 1) * 128, :])
                nc.sync.dma_start(wv_sb[:, kt, :], wv_d[kt * 128:(kt + 1) * 128, :])
                nc.sync.dma_start(wp_sb[:, kt, :], wp_d[kt * 128:(kt + 1) * 128, :])
            nc.sync.dma_start(pb_sb[:], pb_d[:])
            # ones column fused into v (gives softmax sums as O^T row 64)
            nc.vector.memset(v_sb[:, :, :, D:D + 1], 1.0)

            def emit_qkT(ot):
                ps = pspool.tile([128, NCH, 512], f32, tag="ps")
                for nch in range(NCH):
                    for kt in range(CT):
                        nc.tensor.matmul(
                            ps[:, nch, :],
                            wqk_sb[:, kt, ot * 128:(ot + 1) * 128],
                            xT_sb[:, kt, nch * 512:(nch + 1) * 512],
                            start=(kt == 0), stop=(kt == CT - 1),
                        )
                nc.vector.tensor_copy(qkT_sb[:, ot, :, :], ps[:, :, :])

            def emit_v(nt):
                ps = pspool.tile([128, 2, 8, 64], f32, tag="ps")
                for och in range(2):
                    for kt in range(CT):
                        nc.tensor.matmul(
                            ps[:, och, 0:6, :],
                            xT_sb[:, kt, nt * 128:(nt + 1) * 128],
                            wv_sb[:, kt, och * 384:(och + 1) * 384],
                            start=(kt == 0), stop=(kt == CT - 1),
                        )
                    nc.vector.tensor_copy(
                        v_sb[:, nt, och * 6:(och + 1) * 6, 0:D], ps[:, och, 0:6, :]
                    )

            # ---- attention, one head pair at a time. Emission order keeps the
            # in-order PE queue stall-free: S (feeds ScalarE exp stream), then
            # independent filler matmuls (v for pair 0, next pair's qkT after),
            # then O which consumes the exp results.
            for hp in range(HP):
                a, b = 2 * hp, 2 * hp + 1
                if hp == 0:
                    emit_qkT(0)
                    emit_qkT(6)
                E_a = epool.tile([128, NT, NCH, 512], bf16, tag="E")
                E_b = epool.tile([128, NT, NCH, 512], bf16, tag="E")
                # S^T = k @ q^T per m-tile; exp into E
                for mt in range(NT):
                    ps_sa = pspool.tile([128, NCH, 512], f32, tag="ps")
                    ps_sb = pspool.tile([128, NCH, 512], f32, tag="ps")
                    lo = qkT_sb[0:64, 6 + hp, mt // 4, (mt % 4) * 128:(mt % 4 + 1) * 128]
                    hi = qkT_sb[64:128, 6 + hp, mt // 4, (mt % 4) * 128:(mt % 4 + 1) * 128]
                    for nch in range(NCH):
                        nc.tensor.matmul(
                            ps_sa[:, nch, :], lo, qkT_sb[0:64, hp, nch, :],
                            start=True, stop=True,
                        )
                        nc.tensor.matmul(
                            ps_sb[:, nch, :], hi, qkT_sb[64:128, hp, nch, :],
                            start=True, stop=True,
                        )
                    import concourse.mybir as mybir2
                    nc.scalar.activation(
                        E_a[:, mt, :, :], ps_sa[:, :, :],
                        mybir2.ActivationFunctionType.Exp, scale=SCALE,
                    )
                    nc.scalar.activation(
                        E_b[:, mt, :, :], ps_sb[:, :, :],
                        mybir2.ActivationFunctionType.Exp, scale=SCALE,
                    )
                # independent PE filler while ScalarE works through the exps
                if hp == 0:
                    for nt in range(NT):
                        emit_v(nt)
                if hp + 1 < HP:
                    emit_qkT(hp + 1)
                    emit_qkT(6 + hp + 1)
                # O^T = [v|1].T @ E, accumulated over m-tiles
                ps_oa = pspool.tile([65, NCH, 512], f32, tag="ps")
                ps_ob = pspool.tile([65, NCH, 512], f32, tag="ps")
                for mt in range(NT):
                    for nch in range(NCH):
                        nc.tensor.matmul(
                            ps_oa[:, nch, :], v_sb[:, mt, a, :], E_a[:, mt, nch, :],
                            start=(mt == 0), stop=(mt == NT - 1),
                        )
                        nc.tensor.matmul(
                            ps_ob[:, nch, :], v_sb[:, mt, b, :], E_b[:, mt, nch, :],
                            start=(mt == 0), stop=(mt == NT - 1),
                        )
                # normalize: row 64 holds the softmax denominators
                rec_a = spool.tile([1, NCH, 512], f32, tag="rec")
                rec_b = spool.tile([1, NCH, 512], f32, tag="rec")
                sum_a = spool.tile([1, NCH, 512], f32, tag="sum")
                sum_b = spool.tile([1, NCH, 512], f32, tag="sum")
                nc.vector.tensor_copy(sum_a[:], ps_oa[64:65, :, :])
                nc.vector.tensor_copy(sum_b[:], ps_ob[64:65, :, :])
                nc.vector.reciprocal_approx_fast(rec_a[:], sum_a[:])
                nc.vector.reciprocal_approx_fast(rec_b[:], sum_b[:])
                R_a = spool.tile([64, NCH, 512], f32, tag="R")
                R_b = spool.tile([64, NCH, 512], f32, tag="R")
                nc.gpsimd.partition_broadcast(R_a[:], rec_a[:])
                nc.gpsimd.partition_broadcast(R_b[:], rec_b[:])
                import concourse.mybir as mybir3
                nc.vector.tensor_tensor(
                    on_sb[0:64, hp, :, :], ps_oa[0:64, :, :], R_a[:],
                    op=mybir3.AluOpType.mult,
                )
                onb = spool.tile([64, NCH, 512], bf16, tag="onb")
                nc.vector.tensor_tensor(
                    onb[:], ps_ob[0:64, :, :], R_b[:], op=mybir3.AluOpType.mult,
                )
                # head b lives at partitions 64:128 -> shift via SBUF->SBUF DMA
                nc.sync.dma_start(on_sb[64:128, hp, :, :], onb[:])

            # ---- yT = wp.T @ Onorm^T + pb ----
            for otp in range(CT):
                ps = pspool.tile([128, NCH, 512], f32, tag="ps")
                for nch in range(NCH):
                    for kt in range(CT):
                        nc.tensor.matmul(
                            ps[:, nch, :],
                            wp_sb[:, kt, otp * 128:(otp + 1) * 128],
                            on_sb[:, kt, nch, :],
                            start=(kt == 0), stop=(kt == CT - 1),
                        )
                yt = ypool.tile([128, NCH, 512], f32, tag="yt")
                nc.vector.tensor_scalar_add(yt[:], ps[:, :, :], pb_sb[:, otp:otp + 1])
                nc.sync.dma_start(out_d[otp * 128:(otp + 1) * 128, :], yt[:])

    nc.compile()
    return nc


def _get_nc():
    if "nc" not in _CACHE:
        _CACHE["nc"] = _build_nc()
    return _CACHE["nc"]


def kernel(x, qkv_w, proj_w, proj_b):
    from concourse.bass_utils import run_bass_kernel_spmd

    nc = _get_nc()
    bf = ml_dtypes.bfloat16
    wqk = np.ascontiguousarray(qkv_w[:2 * C].T).astype(bf)
    wv = np.ascontiguousarray(qkv_w[2 * C:].T).astype(bf)
    wp = np.ascontiguousarray(proj_w.T).astype(bf)
    pb = np.ascontiguousarray(proj_b.reshape(CT, 128).T).astype(np.float32)
    in_maps = []
    for i in range(B):
        in_maps.append({
            "xT": np.ascontiguousarray(x[i].T).astype(bf),
            "wqk": wqk, "wv": wv, "wp": wp, "pb": pb,
        })
    res = run_bass_kernel_spmd(nc, in_maps, core_ids=list(range(B)))
    out = np.stack([res.results[i]["out"].T for i in range(B)])
    return np.ascontiguousarray(out.astype(np.float32))


# revision 13
# speedup vs baseline: 1.2796x; 1.0538x over previous
"""Multi-head attention (B=8, N=1024, C=768, H=12) on 8 TRN2 NeuronCores.

Sharding: pure data-parallel over batch — core b computes attention for x[b].
No collectives needed. Per-core Bass/Tile kernel, bf16 compute, f32 PSUM.

Layout strategy (all transposes done on host, for free):
  xT  = x[b].T                    [768, 1024]  (c on partitions)
  wqk = qkv_w[:1536].T            [768, 1536]  (c on partitions)
  wv  = qkv_w[1536:].T            [768, 768]
  wp  = proj_w.T                  [768, 768]   (o on partitions)
  pb  = proj_b.reshape(6,128).T   [128, 6]

Device compute per core:
  qkT[o, n]  = wqk.T @ xT         [1536, 1024] (q rows 0:768, k rows 768:1536;
                                   head h occupies partition rows h*64 mod 128
                                   of o-tile h//2; heads are pair-stacked)
  v[n, o]    = xT.T @ wv          [1024, 768]  (natural layout, stored with a
                                   fused ones-column per head -> [nt, h, 65])
  per head:  S^T[m, n] = k q^T (K=64 matmul, two heads row-packed in PE array)
             E = exp(S^T * scale)            (ScalarE, no max subtraction --
                                              |S*scale| <= ~6 for randn inputs)
             O^T[d, n], sums[n] = [v|1].T @ E  (K=128 accumulate over m-tiles;
                                              row 64 = softmax denominators)
             Onorm^T = O^T * (1/sums) broadcast (reciprocal + gpsimd
                                              partition_broadcast)
  yT[o', n]  = wp.T @ Onorm^T + pb           [768, 1024]
Host: out[b] = yT.T
"""

import numpy as np
import ml_dtypes

B, N, C = 8, 1024, 768
H, D = 12, 64
SCALE = D ** -0.5
CT = C // 128       # 6 contraction tiles
OT = 2 * C // 128   # 12 o-tiles of qkT
NT = N // 128       # 8 token tiles
NCH = N // 512      # 2 n-chunks of 512
HP = H // 2         # 6 head pairs

_CACHE = {}


def _build_nc():
    import concourse.bacc as bacc
    import concourse.mybir as mybir
    import concourse.tile as tile

    f32 = mybir.dt.float32
    bf16 = mybir.dt.bfloat16

    nc = bacc.Bacc("TRN2", target_bir_lowering=False, debug=False, num_devices=8)

    xT_d = nc.dram_tensor("xT", [C, N], bf16, kind="ExternalInput").ap()
    wqk_d = nc.dram_tensor("wqk", [C, 2 * C], bf16, kind="ExternalInput").ap()
    wv_d = nc.dram_tensor("wv", [C, C], bf16, kind="ExternalInput").ap()
    wp_d = nc.dram_tensor("wp", [C, C], bf16, kind="ExternalInput").ap()
    pb_d = nc.dram_tensor("pb", [128, CT], f32, kind="ExternalInput").ap()
    out_d = nc.dram_tensor("out", [C, N], f32, kind="ExternalOutput").ap()

    with tile.TileContext(nc) as tc:
        with (
            tc.tile_pool(name="const", bufs=1) as cpool,
            tc.tile_pool(name="E", bufs=3) as epool,
            tc.tile_pool(name="small", bufs=2) as spool,
            tc.tile_pool(name="y", bufs=2) as ypool,
            tc.tile_pool(name="ps", bufs=4, space="PSUM") as pspool,
        ):
            # ---- persistent SBUF tensors ----
            xT_sb = cpool.tile([128, CT, N], bf16)            # 12KB/part
            wqk_sb = cpool.tile([128, CT, 2 * C], bf16)       # 18KB
            wv_sb = cpool.tile([128, CT, C], bf16)            # 9KB
            wp_sb = cpool.tile([128, CT, C], bf16)            # 9KB
            pb_sb = cpool.tile([128, CT], f32)
            qkT_sb = cpool.tile([128, OT, NCH, 512], bf16)    # 24KB
            v_sb = cpool.tile([128, NT, H, D + 1], bf16)      # 12.2KB
            on_sb = cpool.tile([128, CT, NCH, 512], bf16)     # Onorm^T, 12KB

            # DMA order: what the first attention pair needs lands first
            for kt in range(CT):
                nc.sync.dma_start(xT_sb[:, kt, :], xT_d[kt * 128:(kt + 1) * 128, :])
            for kt in range(CT):
                r = kt * 128
                nc.sync.dma_start(wqk_sb[:, kt, 0:128], wqk_d[r:r + 128, 0:128])
                nc.sync.dma_start(wqk_sb[:, kt, 768:896], wqk_d[r:r + 128, 768:896])
            for kt in range(CT):
                r = kt * 128
                nc.sync.dma_start(wv_sb[:, kt, :], wv_d[r:r + 128, :])
                nc.sync.dma_start(wqk_sb[:, kt, 128:768], wqk_d[r:r + 128, 128:768])
                nc.sync.dma_start(wqk_sb[:, kt, 896:1536], wqk_d[r:r + 128, 896:1536])
                nc.sync.dma_start(wp_sb[:, kt, :], wp_d[r:r + 128, :])
            nc.sync.dma_start(pb_sb[:], pb_d[:])
            # ones column fused into v (gives softmax sums as O^T row 64)
            nc.vector.memset(v_sb[:, :, :, D:D + 1], 1.0)

            # kt outer / nch inner: the stationary operand is reused across
            # the two moving chunks, halving LDWEIGHTS pressure
            def emit_qkT(ot):
                ps = pspool.tile([128, NCH, 512], f32, tag="ps")
                for kt in range(CT):
                    for nch in range(NCH):
                        nc.tensor.matmul(
                            ps[:, nch, :],
                            wqk_sb[:, kt, ot * 128:(ot + 1) * 128],
                            xT_sb[:, kt, nch * 512:(nch + 1) * 512],
                            start=(kt == 0), stop=(kt == CT - 1),
                        )
                nc.vector.tensor_copy(qkT_sb[:, ot, :, :], ps[:, :, :])

            def emit_v(nt):
                ps = pspool.tile([128, 2, 8, 64], f32, tag="ps")
                for kt in range(CT):
                    for och in range(2):
                        nc.tensor.matmul(
                            ps[:, och, 0:6, :],
                            xT_sb[:, kt, nt * 128:(nt + 1) * 128],
                            wv_sb[:, kt, och * 384:(och + 1) * 384],
                            start=(kt == 0), stop=(kt == CT - 1),
                        )
                for och in range(2):
                    nc.vector.tensor_copy(
                        v_sb[:, nt, och * 6:(och + 1) * 6, 0:D], ps[:, och, 0:6, :]
                    )

            # ---- attention, one head pair at a time. Emission order keeps the
            # in-order PE queue stall-free: S (feeds ScalarE exp stream), then
            # independent filler matmuls (v for pair 0, next pair's qkT after),
            # then O which consumes the exp results.
            for hp in range(HP):
                a, b = 2 * hp, 2 * hp + 1
                if hp == 0:
                    emit_qkT(0)
                    emit_qkT(6)
                E_a = epool.tile([128, NT, NCH, 512], bf16, tag="E")
                E_b = epool.tile([128, NT, NCH, 512], bf16, tag="E")
                # S^T = k @ q^T per m-tile; exp into E
                for mt in range(NT):
                    ps_sa = pspool.tile([128, NCH, 512], f32, tag="ps")
                    ps_sb = pspool.tile([128, NCH, 512], f32, tag="ps")
                    lo = qkT_sb[0:64, 6 + hp, mt // 4, (mt % 4) * 128:(mt % 4 + 1) * 128]
                    hi = qkT_sb[64:128, 6 + hp, mt // 4, (mt % 4) * 128:(mt % 4 + 1) * 128]
                    for nch in range(NCH):
                        nc.tensor.matmul(
                            ps_sa[:, nch, :], lo, qkT_sb[0:64, hp, nch, :],
                            start=True, stop=True,
                        )
                    for nch in range(NCH):
                        nc.tensor.matmul(
                            ps_sb[:, nch, :], hi, qkT_sb[64:128, hp, nch, :],
                            start=True, stop=True,
                        )
                    import concourse.mybir as mybir2
                    nc.scalar.activation(
                        E_a[:, mt, :, :], ps_sa[:, :, :],
                        mybir2.ActivationFunctionType.Exp, scale=SCALE,
                    )
                    nc.scalar.activation(
                        E_b[:, mt, :, :], ps_sb[:, :, :],
                        mybir2.ActivationFunctionType.Exp, scale=SCALE,
                    )
                # independent PE filler while ScalarE works through the exps
                if hp == 0:
                    for nt in range(NT):
                        emit_v(nt)
                if hp + 1 < HP:
                    emit_qkT(hp + 1)
                    emit_qkT(6 + hp + 1)
                # O^T = [v|1].T @ E, accumulated over m-tiles
                ps_oa = pspool.tile([65, NCH, 512], f32, tag="ps")
                ps_ob = pspool.tile([65, NCH, 512], f32, tag="ps")
                for mt in range(NT):
                    for nch in range(NCH):
                        nc.tensor.matmul(
                            ps_oa[:, nch, :], v_sb[:, mt, a, :], E_a[:, mt, nch, :],
                            start=(mt == 0), stop=(mt == NT - 1),
                        )
                    for nch in range(NCH):
                        nc.tensor.matmul(
                            ps_ob[:, nch, :], v_sb[:, mt, b, :], E_b[:, mt, nch, :],
                            start=(mt == 0), stop=(mt == NT - 1),
                        )
                # normalize: row 64 holds the softmax denominators
                rec_a = spool.tile([1, NCH, 512], f32, tag="rec")
                rec_b = spool.tile([1, NCH, 512], f32, tag="rec")
                sum_a = spool.tile([1, NCH, 512], f32, tag="sum")
                sum_b = spool.tile([1, NCH, 512], f32, tag="sum")
                nc.vector.tensor_copy(sum_a[:], ps_oa[64:65, :, :])
                nc.vector.tensor_copy(sum_b[:], ps_ob[64:65, :, :])
                nc.vector.reciprocal_approx_fast(rec_a[:], sum_a[:])
                nc.vector.reciprocal_approx_fast(rec_b[:], sum_b[:])
                R_a = spool.tile([64, NCH, 512], f32, tag="R")
                R_b = spool.tile([64, NCH, 512], f32, tag="R")
                nc.gpsimd.partition_broadcast(R_a[:], rec_a[:])
                nc.gpsimd.partition_broadcast(R_b[:], rec_b[:])
                import concourse.mybir as mybir3
                nc.vector.tensor_tensor(
                    on_sb[0:64, hp, :, :], ps_oa[0:64, :, :], R_a[:],
                    op=mybir3.AluOpType.mult,
                )
                onb = spool.tile([64, NCH, 512], bf16, tag="onb")
                nc.vector.tensor_tensor(
                    onb[:], ps_ob[0:64, :, :], R_b[:], op=mybir3.AluOpType.mult,
                )
                # head b lives at partitions 64:128 -> shift via SBUF->SBUF DMA
                nc.sync.dma_start(on_sb[64:128, hp, :, :], onb[:])

            # ---- yT = wp.T @ Onorm^T + pb ----
            for otp in range(CT):
                ps = pspool.tile([128, NCH, 512], f32, tag="ps")
                for kt in range(CT):
                    for nch in range(NCH):
                        nc.tensor.matmul(
                            ps[:, nch, :],
                            wp_sb[:, kt, otp * 128:(otp + 1) * 128],
                            on_sb[:, kt, nch, :],
                            start=(kt == 0), stop=(kt == CT - 1),
                        )
                yt = ypool.tile([128, NCH, 512], f32, tag="yt")
                nc.vector.tensor_scalar_add(yt[:], ps[:, :, :], pb_sb[:, otp:otp + 1])
                nc.sync.dma_start(out_d[otp * 128:(otp + 1) * 128, :], yt[:])

    nc.compile()
    return nc


def _get_nc():
    if "nc" not in _CACHE:
        _CACHE["nc"] = _build_nc()
    return _CACHE["nc"]


def kernel(x, qkv_w, proj_w, proj_b):
    from concourse.bass_utils import run_bass_kernel_spmd

    nc = _get_nc()
    bf = ml_dtypes.bfloat16
    wqk = np.ascontiguousarray(qkv_w[:2 * C].T).astype(bf)
    wv = np.ascontiguousarray(qkv_w[2 * C:].T).astype(bf)
    wp = np.ascontiguousarray(proj_w.T).astype(bf)
    pb = np.ascontiguousarray(proj_b.reshape(CT, 128).T).astype(np.float32)
    in_maps = []
    for i in range(B):
        in_maps.append({
            "xT": np.ascontiguousarray(x[i].T).astype(bf),
            "wqk": wqk, "wv": wv, "wp": wp, "pb": pb,
        })
    res = run_bass_kernel_spmd(nc, in_maps, core_ids=list(range(B)))
    out = np.stack([res.results[i]["out"].T for i in range(B)])
    return np.ascontiguousarray(out.astype(np.float32))


# revision 16
# speedup vs baseline: 1.2959x; 1.0127x over previous
"""Multi-head attention (B=8, N=1024, C=768, H=12) on 8 TRN2 NeuronCores.

Sharding: pure data-parallel over batch — core b computes attention for x[b].
No collectives needed. Per-core Bass/Tile kernel, bf16 compute, f32 PSUM.

Layout strategy (all transposes done on host, for free):
  xT  = x[b].T                    [768, 1024]  (c on partitions)
  wqk = qkv_w[:1536].T            [768, 1536]  (c on partitions)
  wv  = qkv_w[1536:].T            [768, 768]
  wp  = proj_w.T                  [768, 768]   (o on partitions)
  pb  = proj_b.reshape(6,128).T   [128, 6]

Device compute per core:
  qkT[o, n]  = wqk.T @ xT         [1536, 1024] (q rows 0:768, k rows 768:1536;
                                   head h occupies partition rows h*64 mod 128
                                   of o-tile h//2; heads are pair-stacked)
  v[n, o]    = xT.T @ wv          [1024, 768]  (natural layout, stored with a
                                   fused ones-column per head -> [nt, h, 65])
  per head:  S^T[m, n] = k q^T (K=64 matmul, two heads row-packed in PE array)
             E = exp(S^T * scale)            (ScalarE, no max subtraction --
                                              |S*scale| <= ~6 for randn inputs)
             O^T[d, n], sums[n] = [v|1].T @ E  (K=128 accumulate over m-tiles;
                                              row 64 = softmax denominators)
             Onorm^T = O^T * (1/sums) broadcast (reciprocal + gpsimd
                                              partition_broadcast)
  yT[o', n]  = wp.T @ Onorm^T + pb           [768, 1024]
Host: out[b] = yT.T
"""

import numpy as np
import ml_dtypes

B, N, C = 8, 1024, 768
H, D = 12, 64
SCALE = D ** -0.5
CT = C // 128       # 6 contraction tiles
OT = 2 * C // 128   # 12 o-tiles of qkT
NT = N // 128       # 8 token tiles
NCH = N // 512      # 2 n-chunks of 512
HP = H // 2         # 6 head pairs

_CACHE = {}


def _build_nc():
    import concourse.bacc as bacc
    import concourse.mybir as mybir
    import concourse.tile as tile

    f32 = mybir.dt.float32
    bf16 = mybir.dt.bfloat16

    nc = bacc.Bacc("TRN2", target_bir_lowering=False, debug=False, num_devices=8)

    xT_d = nc.dram_tensor("xT", [C, N], bf16, kind="ExternalInput").ap()
    wqk_d = nc.dram_tensor("wqk", [C, 2 * C], bf16, kind="ExternalInput").ap()
    wv_d = nc.dram_tensor("wv", [C, C], bf16, kind="ExternalInput").ap()
    wp_d = nc.dram_tensor("wp", [C, C], bf16, kind="ExternalInput").ap()
    pb_d = nc.dram_tensor("pb", [128, CT], f32, kind="ExternalInput").ap()
    out_d = nc.dram_tensor("out", [C, N], f32, kind="ExternalOutput").ap()

    with tile.TileContext(nc) as tc:
        with (
            tc.tile_pool(name="const", bufs=1) as cpool,
            tc.tile_pool(name="E", bufs=3) as epool,
            tc.tile_pool(name="small", bufs=2) as spool,
            tc.tile_pool(name="y", bufs=2) as ypool,
            tc.tile_pool(name="ps", bufs=4, space="PSUM") as pspool,
        ):
            # ---- persistent SBUF tensors ----
            xT_sb = cpool.tile([128, CT, N], bf16)            # 12KB/part
            wqk_sb = cpool.tile([128, CT, 2 * C], bf16)       # 18KB
            wv_sb = cpool.tile([128, CT, C], bf16)            # 9KB
            wp_sb = cpool.tile([128, CT, C], bf16)            # 9KB
            pb_sb = cpool.tile([128, CT], f32)
            qkT_sb = cpool.tile([128, OT, NCH, 512], bf16)    # 24KB
            v_sb = cpool.tile([128, NT, H, D + 1], bf16)      # 12.2KB
            on_sb = cpool.tile([128, CT, NCH, 512], bf16)     # Onorm^T, 12KB

            # DMA order: what the first attention pair needs lands first
            for kt in range(CT):
                r = kt * 128
                nc.sync.dma_start(xT_sb[:, kt, :], xT_d[r:r + 128, :])
                nc.sync.dma_start(wqk_sb[:, kt, 0:128], wqk_d[r:r + 128, 0:128])
                nc.sync.dma_start(wqk_sb[:, kt, 768:896], wqk_d[r:r + 128, 768:896])
            for kt in range(CT):
                r = kt * 128
                nc.sync.dma_start(wv_sb[:, kt, :], wv_d[r:r + 128, :])
                nc.sync.dma_start(wqk_sb[:, kt, 128:768], wqk_d[r:r + 128, 128:768])
                nc.sync.dma_start(wqk_sb[:, kt, 896:1536], wqk_d[r:r + 128, 896:1536])
                nc.sync.dma_start(wp_sb[:, kt, :], wp_d[r:r + 128, :])
            nc.sync.dma_start(pb_sb[:], pb_d[:])
            # ones column fused into v (gives softmax sums as O^T row 64)
            nc.vector.memset(v_sb[:, :, :, D:D + 1], 1.0)

            # kt outer / nch inner: the stationary operand is reused across
            # the two moving chunks, halving LDWEIGHTS pressure
            def emit_qkT(ot):
                ps = pspool.tile([128, NCH, 512], f32, tag="ps")
                for kt in range(CT):
                    for nch in range(NCH):
                        nc.tensor.matmul(
                            ps[:, nch, :],
                            wqk_sb[:, kt, ot * 128:(ot + 1) * 128],
                            xT_sb[:, kt, nch * 512:(nch + 1) * 512],
                            start=(kt == 0), stop=(kt == CT - 1),
                        )
                nc.vector.tensor_copy(qkT_sb[:, ot, :, :], ps[:, :, :])

            def emit_v(nt):
                ps = pspool.tile([128, 2, 8, 64], f32, tag="ps")
                for kt in range(CT):
                    for och in range(2):
                        nc.tensor.matmul(
                            ps[:, och, 0:6, :],
                            xT_sb[:, kt, nt * 128:(nt + 1) * 128],
                            wv_sb[:, kt, och * 384:(och + 1) * 384],
                            start=(kt == 0), stop=(kt == CT - 1),
                        )
                for och in range(2):
                    nc.vector.tensor_copy(
                        v_sb[:, nt, och * 6:(och + 1) * 6, 0:D], ps[:, och, 0:6, :]
                    )

            # ---- attention, one head pair at a time. Emission order keeps the
            # in-order PE queue stall-free: S (feeds ScalarE exp stream), then
            # independent filler matmuls (v for pair 0, next pair's qkT after),
            # then O which consumes the exp results.
            for hp in range(HP):
                a, b = 2 * hp, 2 * hp + 1
                if hp == 0:
                    emit_qkT(0)
                    emit_qkT(6)
                E_a = epool.tile([128, NT, NCH, 512], bf16, tag="E")
                E_b = epool.tile([128, NT, NCH, 512], bf16, tag="E")
                # S^T = k @ q^T per m-tile; exp into E
                for mt in range(NT):
                    ps_sa = pspool.tile([128, NCH, 512], f32, tag="ps")
                    ps_sb = pspool.tile([128, NCH, 512], f32, tag="ps")
                    lo = qkT_sb[0:64, 6 + hp, mt // 4, (mt % 4) * 128:(mt % 4 + 1) * 128]
                    hi = qkT_sb[64:128, 6 + hp, mt // 4, (mt % 4) * 128:(mt % 4 + 1) * 128]
                    for nch in range(NCH):
                        nc.tensor.matmul(
                            ps_sa[:, nch, :], lo, qkT_sb[0:64, hp, nch, :],
                            start=True, stop=True,
                        )
                        nc.tensor.matmul(
                            ps_sb[:, nch, :], hi, qkT_sb[64:128, hp, nch, :],
                            start=True, stop=True,
                        )
                    import concourse.mybir as mybir2
                    nc.scalar.activation(
                        E_a[:, mt, :, :], ps_sa[:, :, :],
                        mybir2.ActivationFunctionType.Exp, scale=SCALE,
                    )
                    nc.scalar.activation(
                        E_b[:, mt, :, :], ps_sb[:, :, :],
                        mybir2.ActivationFunctionType.Exp, scale=SCALE,
                    )
                # independent PE filler while ScalarE works through the exps
                if hp == 0:
                    for nt in range(NT):
                        emit_v(nt)
                if hp + 1 < HP:
                    emit_qkT(hp + 1)
                    emit_qkT(6 + hp + 1)
                # O^T = [v|1].T @ E, accumulated over m-tiles
                ps_oa = pspool.tile([65, NCH, 512], f32, tag="ps")
                ps_ob = pspool.tile([65, NCH, 512], f32, tag="ps")
                for mt in range(NT):
                    for nch in range(NCH):
                        nc.tensor.matmul(
                            ps_oa[:, nch, :], v_sb[:, mt, a, :], E_a[:, mt, nch, :],
                            start=(mt == 0), stop=(mt == NT - 1),
                        )
                    for nch in range(NCH):
                        nc.tensor.matmul(
                            ps_ob[:, nch, :], v_sb[:, mt, b, :], E_b[:, mt, nch, :],
                            start=(mt == 0), stop=(mt == NT - 1),
                        )
                # normalize: row 64 holds the softmax denominators
                rec_a = spool.tile([1, NCH, 512], f32, tag="rec")
                rec_b = spool.tile([1, NCH, 512], f32, tag="rec")
                sum_a = spool.tile([1, NCH, 512], f32, tag="sum")
                sum_b = spool.tile([1, NCH, 512], f32, tag="sum")
                nc.vector.tensor_copy(sum_a[:], ps_oa[64:65, :, :])
                nc.vector.tensor_copy(sum_b[:], ps_ob[64:65, :, :])
                nc.vector.reciprocal_approx_fast(rec_a[:], sum_a[:])
                nc.vector.reciprocal_approx_fast(rec_b[:], sum_b[:])
                R_a = spool.tile([64, NCH, 512], f32, tag="R")
                R_b = spool.tile([64, NCH, 512], f32, tag="R")
                nc.gpsimd.partition_broadcast(R_a[:], rec_a[:])
                nc.gpsimd.partition_broadcast(R_b[:], rec_b[:])
                import concourse.mybir as mybir3
                nc.vector.tensor_tensor(
                    on_sb[0:64, hp, :, :], ps_oa[0:64, :, :], R_a[:],
                    op=mybir3.AluOpType.mult,
                )
                onb = spool.tile([64, NCH, 512], bf16, tag="onb")
                nc.vector.tensor_tensor(
                    onb[:], ps_ob[0:64, :, :], R_b[:], op=mybir3.AluOpType.mult,
                )
                # head b lives at partitions 64:128 -> shift via SBUF->SBUF DMA
                nc.sync.dma_start(on_sb[64:128, hp, :, :], onb[:])

            # ---- yT = wp.T @ Onorm^T + pb ----
            for otp in range(CT):
                ps = pspool.tile([128, NCH, 512], f32, tag="ps")
                for kt in range(CT):
                    for nch in range(NCH):
                        nc.tensor.matmul(
                            ps[:, nch, :],
                            wp_sb[:, kt, otp * 128:(otp + 1) * 128],
                            on_sb[:, kt, nch, :],
                            start=(kt == 0), stop=(kt == CT - 1),
                        )
                yt = ypool.tile([128, NCH, 512], f32, tag="yt")
                nc.scalar.activation(
                    yt[:], ps[:, :, :], mybir.ActivationFunctionType.Identity,
                    bias=pb_sb[:, otp:otp + 1],
                )
                nc.sync.dma_start(out_d[otp * 128:(otp + 1) * 128, :], yt[:])

    nc.compile()
    return nc


def _get_nc():
    if "nc" not in _CACHE:
        _CACHE["nc"] = _build_nc()
    return _CACHE["nc"]


def kernel(x, qkv_w, proj_w, proj_b):
    from concourse.bass_utils import run_bass_kernel_spmd

    nc = _get_nc()
    bf = ml_dtypes.bfloat16
    wqk = np.ascontiguousarray(qkv_w[:2 * C].T).astype(bf)
    wv = np.ascontiguousarray(qkv_w[2 * C:].T).astype(bf)
    wp = np.ascontiguousarray(proj_w.T).astype(bf)
    pb = np.ascontiguousarray(proj_b.reshape(CT, 128).T).astype(np.float32)
    in_maps = []
    for i in range(B):
        in_maps.append({
            "xT": np.ascontiguousarray(x[i].T).astype(bf),
            "wqk": wqk, "wv": wv, "wp": wp, "pb": pb,
        })
    res = run_bass_kernel_spmd(nc, in_maps, core_ids=list(range(B)))
    out = np.stack([res.results[i]["out"].T for i in range(B)])
    return np.ascontiguousarray(out.astype(np.float32))
